# revision 13
# baseline (speedup 1.0000x reference)
"""Multi-head attention (B=8, T=2048, D=512, H=8) on 8 TRN2 NeuronCores.

Sharding: data-parallel over batch — one batch element per core, no
collectives. Host prep (part of shard/unshard): transpose x inputs to
[D, T], cast matmul operands to bf16, pass (1 - mask)^T chunk-major and
duplicated per head-pair, transpose the per-core output y^T back to [T, D].

Per-core algorithm v2c (transposed flash, head-pair row-tiling, QBS=512):
  P1: Q^T, K^T per j-block [128, T] (head 2j in partitions 0-63, 2j+1 in
      64-127), V augmented with a ones column per head (softmax denom).
  P2: per (q-block of 512, head-PAIR j, t2-chunk c):
        S_A^T | S_B^T -> one [128, 1024] PSUM tile: two concurrent
            row-tiled matmuls (head A on PE rows 0-63 -> cols 0-511,
            head B on rows 64-127 -> cols 512-1023)
        P_raw = exp(S^T / 8)        (one ACTIVATE FD=1024)
        P     = P_raw * (1-mask)^T  (one DVE tensor_mul; mask tile is
                                     host-duplicated across the two heads)
        O_aug^T += Vaug_h.T @ P     (per head, [65, 512] PSUM accumulate)
      epilogue: stage O_aug to SBUF (frees PSUM banks fast), reciprocal of
      denominators, DRAM-bounce partition broadcast, normalize into o2.
  P3: y^T = Wo^T.T @ O^T (+bo) per (qb, i) tile, DMA out.

Scheduling: projection / V / P3 work is split into ~2-matmul pieces placed
into specific (qb, pair, chunk) slots by a static deadline-aware schedule
so the scalar engine (exp) never starves. Each x input and weight matrix
loads as ONE strided DMA ([128, 4T] / [128, 4D] tiles); mask DMAs issue
from the GpSimd queue to keep the sync queue short. PSUM (8 banks):
s [128,1024]x2 = 4, oA/oB [65,512] = 2, px [128,512]x2 = 2; the tail P3
tiles reuse the freed s-tag banks.
"""

import numpy as np
import ml_dtypes

B, T, FDIM, H = 8, 2048, 512, 8
DK = FDIM // H          # 64
NFT = FDIM // 128       # 4 fo-tiles
NCH = T // 128          # 16 t2-chunks
QB = 4                  # q blocks
QBS = T // QB           # 512
NP = H // 2             # 4 head-pairs
N_CORES = 8

BF16 = ml_dtypes.bfloat16

_cache = {}


def _build_nc():
    import concourse.bass as bass
    import concourse.mybir as mybir
    from concourse import bacc, tile

    f32 = mybir.dt.float32
    bf16 = mybir.dt.bfloat16
    Exp = mybir.ActivationFunctionType.Exp
    Alu = mybir.AluOpType

    nc = bacc.Bacc("TRN2", target_bir_lowering=False, debug=False,
                   num_devices=N_CORES)

    xqT = nc.dram_tensor("xqT", [FDIM, T], bf16, kind="ExternalInput")
    xkT = nc.dram_tensor("xkT", [FDIM, T], bf16, kind="ExternalInput")
    xvT = nc.dram_tensor("xvT", [FDIM, T], bf16, kind="ExternalInput")
    wqT = nc.dram_tensor("wqT", [FDIM, FDIM], bf16, kind="ExternalInput")
    wkT = nc.dram_tensor("wkT", [FDIM, FDIM], bf16, kind="ExternalInput")
    wvT = nc.dram_tensor("wvT", [FDIM, FDIM], bf16, kind="ExternalInput")
    woT = nc.dram_tensor("woT", [FDIM, FDIM], bf16, kind="ExternalInput")
    bq = nc.dram_tensor("bq", [FDIM], f32, kind="ExternalInput")
    bk = nc.dram_tensor("bk", [FDIM], f32, kind="ExternalInput")
    bv = nc.dram_tensor("bv", [FDIM], f32, kind="ExternalInput")
    bo = nc.dram_tensor("bo", [FDIM], f32, kind="ExternalInput")
    # duplicated (1-mask)^T: [c, p, qb*1024 + a*512 + q] with a = head slot
    mbar = nc.dram_tensor("mbar", [NCH, 128, 2 * T], bf16, kind="ExternalInput")
    yT = nc.dram_tensor("yT", [FDIM, T], f32, kind="ExternalOutput")
    rscratch = nc.dram_tensor("rscratch", [2 * QB * NP, QBS], f32)

    with tile.TileContext(nc) as tc:
        with (
            tc.tile_pool(name="consts", bufs=1) as consts,
            tc.tile_pool(name="qt", bufs=1) as qt_pool,
            tc.tile_pool(name="kt", bufs=1) as kt_pool,
            tc.tile_pool(name="vaug", bufs=1) as vaug_pool,
            tc.tile_pool(name="osb", bufs=1) as osb_pool,
            tc.tile_pool(name="ysb", bufs=1) as ysb_pool,
            tc.tile_pool(name="xt", bufs=1) as xt_pool,
            tc.tile_pool(name="mask", bufs=20) as mask_pool,
            tc.tile_pool(name="praw", bufs=3) as praw_pool,
            tc.tile_pool(name="pm", bufs=3) as pm_pool,
            tc.tile_pool(name="rb", bufs=1) as rb_pool,
            tc.tile_pool(name="psum", bufs=1, space="PSUM") as psum_pool,
        ):
            wq_sb = [consts.tile([128, FDIM], bf16, tag=f"wq{fc}", name=f"wq{fc}") for fc in range(4)]
            wk_sb = [consts.tile([128, FDIM], bf16, tag=f"wk{fc}", name=f"wk{fc}") for fc in range(4)]
            wv_sb = [consts.tile([128, FDIM], bf16, tag=f"wv{fc}", name=f"wv{fc}") for fc in range(4)]
            wo_sb = [consts.tile([128, FDIM], bf16, tag=f"wo{j}", name=f"wo{j}") for j in range(NFT)]
            bq_sb = consts.tile([128, NFT], f32, tag="bq", name="bq")
            bk_sb = consts.tile([128, NFT], f32, tag="bk", name="bk")
            bo_sb = consts.tile([128, NFT], f32, tag="bo", name="bo")
            bv_bcast = consts.tile([128, FDIM], f32, tag="bv_bcast", name="bv_bcast")

            xts_k = [xt_pool.tile([128, T], bf16, tag=f"xk{fc}", name=f"xk{fc}") for fc in range(4)]
            xts_q = [xt_pool.tile([128, T], bf16, tag=f"xq{fc}", name=f"xq{fc}") for fc in range(4)]
            xts_v = [xt_pool.tile([128, T], bf16, tag=f"xv{fc}", name=f"xv{fc}") for fc in range(4)]

            qT_sb = [qt_pool.tile([128, T], bf16, tag=f"qT{j}", name=f"qT{j}") for j in range(NFT)]
            kT_sb = [kt_pool.tile([128, T], bf16, tag=f"kT{j}", name=f"kT{j}") for j in range(NFT)]
            vaug = [vaug_pool.tile([128, H * (DK + 1)], bf16, tag=f"va{tt}", name=f"va{tt}")
                    for tt in range(NCH)]
            o2_sb = {}
            for qb in range(QB):
                for j in range(NP):
                    o2_sb[(qb, j)] = osb_pool.tile([128, QBS], bf16, tag=f"o2_{qb}_{j}",
                                                   name=f"o2_{qb}_{j}")

            # ---- DMA issues: K/Q path first; x loads spread over the
            # idle scalar/vector engine DMA queues for parallel streams ----
            for fc in range(4):
                nc.scalar.dma_start(out=xts_k[fc][:], in_=xkT[fc * 128:(fc + 1) * 128, :])
                nc.scalar.dma_start(out=xts_q[fc][:], in_=xqT[fc * 128:(fc + 1) * 128, :])
                nc.sync.dma_start(out=wk_sb[fc][:], in_=wkT[fc * 128:(fc + 1) * 128, :])
            for fc in range(4):
                nc.sync.dma_start(out=wq_sb[fc][:], in_=wqT[fc * 128:(fc + 1) * 128, :])
                nc.sync.dma_start(out=wv_sb[fc][:], in_=wvT[fc * 128:(fc + 1) * 128, :])
                nc.scalar.dma_start(out=xts_v[fc][:], in_=xvT[fc * 128:(fc + 1) * 128, :])
            for b_dram, b_t in ((bq, bq_sb), (bk, bk_sb), (bo, bo_sb)):
                nc.sync.dma_start(out=b_t[:],
                                  in_=b_dram.ap().rearrange("(j p) -> p j", p=128))
            nc.sync.dma_start(
                out=bv_bcast[:],
                in_=bv.ap().rearrange("(a f) -> a f", a=1).to_broadcast([128, FDIM]))
            for j in range(NFT):
                nc.sync.dma_start(out=wo_sb[j][:], in_=woT[j * 128:(j + 1) * 128, :])
            for tt in range(NCH):
                va = vaug[tt][:].rearrange("p (h d) -> p h d", d=DK + 1)
                nc.vector.memset(va[:, :, DK:DK + 1], 1.0)

            # ---- work pieces ----
            gstate = {}

            def v_mm(tt, fcs, first, last):
                def fn():
                    if first:
                        gstate[("v", tt)] = psum_pool.tile(
                            [128, 512], f32, tag="px", bufs=2, name="vp")
                    ps = gstate[("v", tt)]
                    for fc in fcs:
                        nc.tensor.matmul(
                            ps[:], xts_v[fc][:, tt * 128:(tt + 1) * 128],
                            wv_sb[fc][:],
                            start=(fc == 0), stop=(fc == 3))
                    if last:
                        ps = gstate.pop(("v", tt))
                        va = vaug[tt][:].rearrange("p (h d) -> p h d", d=DK + 1)
                        nc.vector.scalar_tensor_tensor(
                            out=va[:, :, 0:DK],
                            in0=ps[:].rearrange("p (h d) -> p h d", d=DK),
                            scalar=1.0,
                            in1=bv_bcast[:].rearrange("p (h d) -> p h d", d=DK),
                            op0=Alu.mult, op1=Alu.add)
                return fn

            def pj_mm(kind, j, s, fcs, first, last):
                def fn():
                    key = (kind, j, s)
                    if first:
                        gstate[key] = psum_pool.tile(
                            [128, 512], f32, tag="px", bufs=2, name="qkp")
                    ps = gstate[key]
                    xts, w_sb = (xts_q, wq_sb) if kind == "q" else (xts_k, wk_sb)
                    for fc in fcs:
                        nc.tensor.matmul(
                            ps[:],
                            w_sb[fc][:, j * 128:(j + 1) * 128],
                            xts[fc][:, s * 512:(s + 1) * 512],
                            start=(fc == 0), stop=(fc == 3))
                    if last:
                        ps = gstate.pop(key)
                        b_t, dst = ((bq_sb, qT_sb[j]) if kind == "q"
                                    else (bk_sb, kT_sb[j]))
                        nc.vector.tensor_scalar_add(
                            dst[:, s * 512:(s + 1) * 512], ps[:], b_t[:, j:j + 1])
                return fn

            def p3_mm(qb, i, jjs, first, last, tag="px"):
                def fn():
                    key = ("p3", qb, i)
                    if first:
                        gstate[key] = psum_pool.tile(
                            [128, QBS], f32, tag=tag, bufs=2, name="y")
                    y_ps = gstate[key]
                    for jj in jjs:
                        nc.tensor.matmul(
                            y_ps[:], wo_sb[jj][:, i * 128:(i + 1) * 128],
                            o2_sb[(qb, jj)][:],
                            start=(jj == 0), stop=(jj == NFT - 1))
                    if last:
                        y_ps = gstate.pop(key)
                        y_sb = ysb_pool.tile([128, QBS], f32, tag="ysb",
                                             bufs=2, name="ysb")
                        nc.vector.tensor_scalar_add(y_sb[:], y_ps[:],
                                                    bo_sb[:, i:i + 1])
                        nc.sync.dma_start(
                            out=yT[i * 128:(i + 1) * 128, qb * QBS:(qb + 1) * QBS],
                            in_=y_sb[:])
                return fn

            mask_t = {qb: [None] * NCH for qb in range(QB)}

            def m_load(qb, c):
                def fn():
                    mt = mask_pool.tile([128, 2 * QBS], bf16, tag="mask",
                                        name="mask")
                    nc.sync.dma_start(
                        out=mt[:],
                        in_=mbar[c, :, qb * 2 * QBS:(qb + 1) * 2 * QBS])
                    mask_t[qb][c] = mt
                return fn

            def Ga(kind, j, s):
                return (v_mm(j, (0, 1), True, False) if kind == "v"
                        else pj_mm(kind, j, s, (0, 1), True, False))

            def Gb(kind, j, s):
                return (v_mm(j, (2, 3), False, True) if kind == "v"
                        else pj_mm(kind, j, s, (2, 3), False, True))

            # ---- static schedule: (qb, j, c) -> pieces ----
            SCHED = {}

            def at(qb, j, c, *fns):
                SCHED.setdefault((qb, j, c), []).extend(fns)

            def put_group(qb, j, c0, kind, jj, s):
                at(qb, j, c0, Ga(kind, jj, s))
                at(qb, j, c0 + 1, Gb(kind, jj, s))

            # (0,0): V8..15 (vaug[tt] due at chunk tt) + K1s0 + Q1s0
            for tt in range(8, 16):
                put_group(0, 0, 2 * (tt - 8), "v", tt, 0)
            at(0, 0, 1, Ga("k", 1, 0)); at(0, 0, 3, Gb("k", 1, 0))
            at(0, 0, 5, Ga("q", 1, 0)); at(0, 0, 7, Gb("q", 1, 0))
            # (0,1..3): K(j)s1-3 due at chunks 4/8/12; next pair's K s0+Q s0
            for j in range(1, 4):
                put_group(0, j, 0, "k", j, 1)
                put_group(0, j, 4, "k", j, 2)
                put_group(0, j, 8, "k", j, 3)
                if j < 3:
                    put_group(0, j, 10, "k", j + 1, 0)
                    put_group(0, j, 12, "q", j + 1, 0)
            # (0,3): Q s1 prep for qb1
            put_group(0, 3, 10, "q", 0, 1)
            put_group(0, 3, 12, "q", 1, 1)
            put_group(1, 0, 0, "q", 2, 1)
            put_group(1, 0, 2, "q", 3, 1)
            # Q s2 during (1,2)/(1,3), Q s3 during (2,2)/(2,3)
            for qb in (1, 2):
                put_group(qb, 2, 0, "q", 0, qb + 1)
                put_group(qb, 2, 4, "q", 1, qb + 1)
                put_group(qb, 3, 0, "q", 2, qb + 1)
                put_group(qb, 3, 4, "q", 3, qb + 1)
            # P3(qb-1) during (qb,0)/(qb,1)
            for qb in (1, 2, 3):
                for i in range(4):
                    jloc, cbase = (0, 5) if i < 2 else (1, 1)
                    c0 = cbase + 4 * (i % 2)
                    at(qb, jloc, c0, p3_mm(qb - 1, i, (0, 1), True, False))
                    at(qb, jloc, c0 + 1, p3_mm(qb - 1, i, (2, 3), False, True))
            # P3(3) i=0,1: incremental j-accumulation as o2(3,j) lands
            at(3, 1, 9, p3_mm(3, 0, (0,), True, False))
            at(3, 1, 11, p3_mm(3, 1, (0,), True, False))
            at(3, 2, 2, p3_mm(3, 0, (1,), False, False))
            at(3, 2, 4, p3_mm(3, 1, (1,), False, False))
            at(3, 3, 2, p3_mm(3, 0, (2,), False, False))
            at(3, 3, 4, p3_mm(3, 1, (2,), False, False))
            # mask DMA issues
            for qb in range(QB):
                for k in range(4):
                    at(qb, 0, k, m_load(qb, 4 + k))
                for k in range(8):
                    at(qb, 0, 4 + k, m_load(qb, 8 + k))
                if qb < QB - 1:
                    for k in range(4):
                        at(qb, 3, k, m_load(qb + 1, k))

            def epilogue_pair(o_psA, o_psB, qb, j):
                oSA = rb_pool.tile([DK + 1, QBS], f32, tag="oSA", bufs=2,
                                   name="oSA")
                oSB = rb_pool.tile([DK + 1, QBS], f32, tag="oSB", bufs=2,
                                   name="oSB")
                nc.vector.tensor_copy(oSA[:], o_psA[:])
                nc.vector.tensor_copy(oSB[:], o_psB[:])
                rbs = rb_pool.tile([16, QBS // 8], f32, tag="rbs", name="rbs")
                rbr = rb_pool.tile([16, QBS // 8], f32, tag="rbr", name="rbr")
                nc.sync.dma_start(out=rbs[0:8, :], in_=oSA[DK:DK + 1, :])
                nc.sync.dma_start(out=rbs[8:16, :], in_=oSB[DK:DK + 1, :])
                nc.vector.reciprocal(rbr[:], rbs[:])
                rrows = rscratch.ap()[2 * (qb * NP + j):2 * (qb * NP + j) + 2, :]
                nc.sync.dma_start(
                    out=rrows.rearrange("h (e q) -> h e q", e=8), in_=rbr[:])
                rbb = rb_pool.tile([64, 2 * QBS], f32, tag="rbb", bufs=2,
                                   name="rbb")
                nc.sync.dma_start(
                    out=rbb[:].rearrange("p (h q) -> p h q", h=2),
                    in_=rrows.rearrange("(a h) q -> a h q", a=1)
                    .to_broadcast([64, 2, QBS]))
                nc.vector.tensor_mul(o2_sb[(qb, j)][0:64, :], oSA[0:DK, :],
                                     rbb[:, 0:QBS])
                osm = rb_pool.tile([64, QBS], bf16, tag="osm", bufs=2,
                                   name="osm")
                nc.vector.tensor_mul(osm[:], oSB[0:DK, :], rbb[:, QBS:2 * QBS])
                nc.sync.dma_start(out=o2_sb[(qb, j)][64:128, :], in_=osm[:])

            # ---- P1 prefix: first-QK path first, then V, interleaved ----
            for c in range(4):
                m_load(0, c)()
            Ga("k", 0, 0)(); Gb("k", 0, 0)()
            Ga("q", 0, 0)(); Gb("q", 0, 0)()
            order = [("v", 0), ("v", 1), ("k", 1), ("v", 2), ("v", 3),
                     ("k", 2), ("v", 4), ("v", 5), ("k", 3), ("v", 6),
                     ("v", 7)]
            for kind, idx in order:
                if kind == "v":
                    Ga("v", idx, 0)(); Gb("v", idx, 0)()
                else:
                    Ga("k", 0, idx)(); Gb("k", 0, idx)()

            # ---- P2 ----
            for qb in range(QB):
                qof = qb * QBS
                for j in range(NP):
                    hA, hB = 2 * j, 2 * j + 1
                    o_psA = psum_pool.tile([DK + 1, QBS], f32, tag="oA",
                                           bufs=1, name="oA")
                    o_psB = psum_pool.tile([DK + 1, QBS], f32, tag="oB",
                                           bufs=1, name="oB")
                    for c in range(NCH):
                        for fn in SCHED.get((qb, j, c), ()):
                            fn()
                        s_pair = psum_pool.tile([128, 2 * QBS], f32, tag="s",
                                                bufs=2, name="s_pair")
                        for hh in range(2):
                            rows = slice(64 * hh, 64 * hh + 64)
                            nc.tensor.matmul(
                                s_pair[:, hh * QBS:(hh + 1) * QBS],
                                kT_sb[j][rows, c * 128:(c + 1) * 128],
                                qT_sb[j][rows, qof:qof + QBS],
                                start=True, stop=True)
                        p_raw = praw_pool.tile([128, 2 * QBS], bf16,
                                               tag="praw", name="praw")
                        nc.scalar.activation(p_raw[:], s_pair[:], Exp,
                                             bias=0.0, scale=0.125)
                        p_m = pm_pool.tile([128, 2 * QBS], bf16, tag="pm",
                                           name="pm")
                        nc.vector.tensor_mul(p_m[:], p_raw[:], mask_t[qb][c][:])
                        for hh, h, o_ps in ((0, hA, o_psA), (1, hB, o_psB)):
                            nc.tensor.matmul(
                                o_ps[:],
                                vaug[c][:, h * (DK + 1):(h + 1) * (DK + 1)],
                                p_m[:, hh * QBS:(hh + 1) * QBS],
                                start=(c == 0), stop=(c == NCH - 1))
                    epilogue_pair(o_psA, o_psB, qb, j)

            # ---- tail: y2/y3 members that need only o2(3,0..2) first,
            # then the four o2(3,3)-gated j3 finishes ----
            for i in (2, 3):
                p3_mm(3, i, (0, 1), True, False, tag="s")()
            for i in (2, 3):
                p3_mm(3, i, (2,), False, False, tag="s")()
            p3_mm(3, 0, (3,), False, True)()
            p3_mm(3, 1, (3,), False, True)()
            for i in (2, 3):
                p3_mm(3, i, (3,), False, True, tag="s")()

    nc.compile()
    return nc


def _get_nc():
    if "nc" not in _cache:
        _cache["nc"] = _build_nc()
    return _cache["nc"]


def _make_in_maps(inputs):
    query = np.asarray(inputs["query"], np.float32)
    key = np.asarray(inputs["key"], np.float32)
    value = np.asarray(inputs["value"], np.float32)
    mask = np.asarray(inputs["mask"], bool)
    shared = {
        "wqT": np.ascontiguousarray(np.asarray(inputs["Wq"], np.float32).T).astype(BF16),
        "wkT": np.ascontiguousarray(np.asarray(inputs["Wk"], np.float32).T).astype(BF16),
        "wvT": np.ascontiguousarray(np.asarray(inputs["Wv"], np.float32).T).astype(BF16),
        "woT": np.ascontiguousarray(np.asarray(inputs["Wo"], np.float32).T).astype(BF16),
        "bq": np.asarray(inputs["bq"], np.float32),
        "bk": np.asarray(inputs["bk"], np.float32),
        "bv": np.asarray(inputs["bv"], np.float32),
        "bo": np.asarray(inputs["bo"], np.float32),
    }
    in_maps = []
    for b in range(N_CORES):
        m = dict(shared)
        m["xqT"] = np.ascontiguousarray(query[b].T).astype(BF16)
        m["xkT"] = np.ascontiguousarray(key[b].T).astype(BF16)
        m["xvT"] = np.ascontiguousarray(value[b].T).astype(BF16)
        mb = (~mask[b]).T.astype(BF16).reshape(NCH, 128, QB, 1, QBS)
        m["mbar"] = np.ascontiguousarray(
            np.broadcast_to(mb, (NCH, 128, QB, 2, QBS)).reshape(NCH, 128, 2 * T))
        in_maps.append(m)
    return in_maps


def run(inputs, trace=False, **kwargs):
    from concourse.bass_utils import run_bass_kernel_spmd
    nc = _get_nc()
    res = run_bass_kernel_spmd(nc, _make_in_maps(inputs),
                               core_ids=list(range(N_CORES)),
                               trace=trace, **kwargs)
    y = np.stack([np.asarray(res.results[b]["yT"], np.float32).T
                  for b in range(N_CORES)])
    return y, res


def kernel(**inputs) -> np.ndarray:
    y, _ = run(inputs, trace=False)
    return y
